# revision 20
# baseline (speedup 1.0000x reference)
"""DeformableConv1d TRN2 Bass kernel (v6).

Per batch sample (one NeuronCore each, 8 cores):
  offset/mask = conv1d over x.T; pos = clip(l+off); fl/alpha; out[c,l] =
  sum_{s=-3..3} vv_s[l] * x[l+s, c]  (7-diagonal band).

v6 = v4 PE structure (6-matmul conv accumulate, zero-init band) plus:
 - M2 band matrix via gpsimd.local_scatter (replaces copy_predicated +
   Act permutes + memsets): ~5us on the idle Pool engine.
 - bf16 output stores (host casts back to fp32): halves store traffic.
 - stores staged 1024 wide and spread over sync/scalar/gpsimd queues.
 - sigmoid Act table preloaded at boot.
 - shifts_chunk(1) hoisted before the first band banks so the last two
   local_scatters overlap band matmuls instead of stalling them.
"""
import numpy as np
from contextlib import ExitStack

import bass_rust
import ml_dtypes
import concourse.bacc as bacc
import concourse.bass as bass
import concourse.tile as tile
from concourse import mybir
from concourse.bass_utils import run_bass_kernel_spmd

AP = bass_rust.AP
dt = mybir.dt
F32 = dt.float32
BF16 = dt.bfloat16
BF = ml_dtypes.bfloat16

B, L, C, K = 8, 4096, 256, 3
P = 128
NT = L // P            # 32 l-tiles
ND = 7                 # diagonals s in [-3, 3]
F = 134                # band window per tile: f in [0,134), l = 128m-3+f
XT_W = L + 4           # xT: data at cols [2, L+2), zeros at edges
M0_END = 15            # m-chunk boundary (chunk1 needs vv2[m=16] carry)
_cache = {}


def _build(w_off, b_off, w_mask, b_mask):
    nc = bacc.Bacc("TRN2", target_bir_lowering=False, debug=False)

    x_in = nc.dram_tensor("x", [L, C], F32, kind="ExternalInput").ap()
    out_d = nc.dram_tensor("out", [C, L], BF16, kind="ExternalOutput").ap()

    # conv weights [c-in-group, (g, dk, j)]; j<3 offset o, j>=3 mask o
    wcat = np.zeros((P, 36), BF)
    for g in range(2):
        for dkk in range(3):
            for j in range(6):
                w = w_off if j < 3 else w_mask
                wcat[:, g * 18 + dkk * 6 + j] = w[j % 3, g * P:(g + 1) * P, dkk].astype(BF)
    wcat_h = nc.inline_tensor(np.ascontiguousarray(wcat), name="wcat")
    ident_h = nc.inline_tensor(np.eye(P, dtype=BF), name="ident")
    ident6_h = nc.inline_tensor(np.eye(6, dtype=BF), name="ident6")

    # shift matrices (fp8, 0/1 exact): SH_u[k,p]=1[k=p+u-3]; carries wrap
    F8 = ml_dtypes.float8_e4m3
    shmats = {}
    for u in range(ND):
        sh = u - 3
        m_ = np.zeros((P, P), F8)
        for p in range(P):
            if 0 <= p + sh < P:
                m_[p + sh, p] = 1.0
        shmats[("m", u)] = m_
        if sh > 0:
            c_ = np.zeros((P, P), F8)
            for p in range(P - sh, P):
                c_[p + sh - P, p] = 1.0
            shmats[("c", u)] = c_
        elif sh < 0:
            c_ = np.zeros((P, P), F8)
            for p in range(0, -sh):
                c_[p + sh + P, p] = 1.0
            shmats[("c", u)] = c_
    sh_h = {k: nc.inline_tensor(v, name=f"sh_{k[0]}{k[1]}") for k, v in shmats.items()}

    # local_scatter index table: idx[p, mc*8 + u] = mc*F + p + u (u<7), -1 pad
    idxt = np.full((P, 9 * 8), -1, np.int16)
    for p in range(P):
        for mc in range(9):
            for u in range(ND):
                idxt[p, mc * 8 + u] = mc * F + p + u
    idxt_h = nc.inline_tensor(np.ascontiguousarray(idxt), name="idxt")

    # spat_exp[p, fc*112 + si*16 + mc] = si-3 (fc=0 floor) / si-4 (fc=1 ceil)
    spat = np.zeros((P, 2 * ND * 16), BF)
    for fc in range(2):
        for si in range(ND):
            spat[:, fc * 112 + si * 16:fc * 112 + (si + 1) * 16] = si - 3 - fc
    spat_h = nc.inline_tensor(np.ascontiguousarray(spat), name="spatx")

    bo3 = np.tile(np.asarray(b_off, np.float32)[None, :], (P, 1))
    bm3 = np.tile(np.asarray(b_mask, np.float32)[None, :], (P, 1))
    bo3_h = nc.inline_tensor(np.ascontiguousarray(bo3), name="bo3")
    bm3_h = nc.inline_tensor(np.ascontiguousarray(bm3), name="bm3")
    A = mybir.AluOpType

    with tile.TileContext(nc) as tc, ExitStack() as ctx:
        pool = ctx.enter_context(tc.tile_pool(name="main", bufs=1))
        stg = ctx.enter_context(tc.tile_pool(name="stg", bufs=3))
        ps_tr = ctx.enter_context(tc.tile_pool(name="ps_tr", bufs=2, space="PSUM"))
        ps_cv = ctx.enter_context(tc.tile_pool(name="ps_cv", bufs=2, space="PSUM"))
        ps_sh = ctx.enter_context(tc.tile_pool(name="ps_sh", bufs=1, space="PSUM"))
        ps_bd = ctx.enter_context(tc.tile_pool(name="ps_bd", bufs=3, space="PSUM"))

        # ---- x: quads 0-3 via HWDGE fp32 + DVE casts; quads 4-7 via
        #      gpsimd SWDGE cast-DMAs. Small hot consts go first. ----
        xbf8 = [pool.tile([P, 4 * C], BF16, tag=f"xb{q}", name=f"xb{q}")
                for q in range(8)]

        xal0 = pool.tile([P, 4 * C], F32, tag="xal0")
        for hh, eng in ((0, nc.sync), (1, nc.scalar)):
            src = AP(x_in.tensor, hh * 2 * P * C, [[C, P], [P * C, 2], [1, C]])
            dst = AP(xal0[:].tensor, hh * 2 * C, [[4 * C, P], [C, 2], [1, C]])
            eng.dma_start(dst, src)
            nc.vector.tensor_copy(xbf8[0][:, hh * 512:(hh + 1) * 512],
                                  xal0[:, hh * 512:(hh + 1) * 512])
        xal1 = pool.tile([P, 4 * C], F32, tag="xal1")
        src = AP(x_in.tensor, 4 * P * C, [[C, P], [P * C, 4], [1, C]])
        dst = AP(xal1[:].tensor, 0, [[4 * C, P], [C, 4], [1, C]])
        nc.scalar.dma_start(dst, src)
        nc.vector.tensor_copy(xbf8[1][:], xal1[:])

        # gpsimd front-matter BEFORE its dma issues so nothing blocks on drain
        iota = pool.tile([P, NT], F32, tag="iota")
        nc.gpsimd.iota(iota[:], pattern=[[P, NT]], base=0, channel_multiplier=1,
                       allow_small_or_imprecise_dtypes=True)
        zeros_b = pool.tile([P, 512], BF16, tag="zerob")
        nc.gpsimd.memset(zeros_b[:], 0.0)
        xT = [pool.tile([P, XT_W], BF16, tag=f"xT{g}", name=f"xT{g}")
              for g in range(2)]
        for g in range(2):
            nc.vector.memset(xT[g][:, 0:2], 0.0)
            nc.vector.memset(xT[g][:, XT_W - 2:XT_W], 0.0)

        for q in range(2, 8):
            src = AP(x_in.tensor, q * 4 * P * C, [[C, P], [P * C, 4], [1, C]])
            dst = AP(xbf8[q][:].tensor, 0, [[4 * C, P], [C, 4], [1, C]])
            nc.gpsimd.dma_start(dst, src)

        def xsl(m, g):          # x tile m, c-group g: [128 l, 128 c] bf16
            q, t = m // 4, m % 4
            return xbf8[q][:, t * C + g * P: t * C + g * P + P]

        # ---- consts ----
        wcat_s = pool.tile([P, 36], BF16, tag="wcat")
        nc.scalar.dma_start(wcat_s[:], wcat_h.ap())
        ident_s = pool.tile([P, P], BF16, tag="ident")
        nc.sync.dma_start(ident_s[:], ident_h.ap())
        ident6_s = pool.tile([6, 6], BF16, tag="ident6")
        nc.scalar.dma_start(ident6_s[:], ident6_h.ap())
        sh_s = {}
        for kk, h in sh_h.items():
            t_ = pool.tile([P, P], dt.float8e4, tag=f"sh_{kk[0]}{kk[1]}",
                           name=f"sh_{kk[0]}{kk[1]}")
            nc.sync.dma_start(t_[:], h.ap())
            sh_s[kk] = t_
        idxt_s = pool.tile([P, 9 * 8], dt.int16, tag="idxt")
        nc.scalar.dma_start(idxt_s[:], idxt_h.ap())
        spat_s = pool.tile([P, 2 * ND * 16], BF16, tag="spatx")
        nc.sync.dma_start(spat_s[:], spat_h.ap())
        bo3_s = pool.tile([P, 3], F32, tag="bo3")
        nc.scalar.dma_start(bo3_s[:], bo3_h.ap())
        bm3_s = pool.tile([P, 3], F32, tag="bm3")
        nc.sync.dma_start(bm3_s[:], bm3_h.ap())

        # preload the sigmoid Act table at boot (avoids mid-kernel table swap)
        sgld = pool.tile([1, 8], F32, tag="sgld")
        nc.scalar.activation(sgld[:], iota[0:1, 0:8],
                             mybir.ActivationFunctionType.Sigmoid)

        z6 = pool.tile([6, L], BF16, tag="z6")
        zT6 = pool.tile([P, NT * 6], BF16, tag="zT6")

        # ---- transposes: 4 per psum tile per g; copy -> xT ----
        def tr_quad(Q):         # tiles 4Q..4Q+3
            for g in range(2):
                pt = ps_tr.tile([P, 512], BF16, tag="pt")
                for i in range(4):
                    m = 4 * Q + i
                    nc.tensor.transpose(pt[:, i * P:(i + 1) * P],
                                        xsl(m, g), ident_s[:])
                dst = xT[g][:, 2 + Q * 512: 2 + (Q + 1) * 512]
                if (Q + g) % 2 == 0:
                    nc.vector.tensor_copy(dst, pt[:])
                else:
                    nc.scalar.copy(dst, pt[:])

        def conv_chunk(chk):
            pz = ps_cv.tile([6, 512], F32, tag="pz")
            n = 0
            for g in range(2):
                for dkk in range(3):
                    lhsT = wcat_s[:, g * 18 + dkk * 6: g * 18 + dkk * 6 + 6]
                    rhs = xT[g][:, chk * 512 + dkk + 1: chk * 512 + dkk + 513]
                    nc.tensor.matmul(pz[:], lhsT, rhs, start=(n == 0), stop=(n == 5))
                    n += 1
            if chk % 2 == 0:
                nc.vector.tensor_copy(z6[:, chk * 512:(chk + 1) * 512], pz[:])
            else:
                nc.scalar.copy(z6[:, chk * 512:(chk + 1) * 512], pz[:])

        # ---- per-half: zT6, elementwise, VV2 ----
        vv2 = pool.tile([P, ND * NT], BF16, tag="vv2")
        dwb = [pool.tile([P, 3 * 16 * 3], BF16, tag=f"dwb{h}", name=f"dwb{h}")
               for h in range(2)]  # per half: [d | wf | wc] each [p, o*16+mc]

        def half_front(h):
            mr0 = h * 16
            pzt = ps_cv.tile([P, 96], BF16, tag="pz", name=f"pzt{h}")
            for i in range(16):
                m = mr0 + i
                nc.tensor.transpose(pzt[:, i * 6:(i + 1) * 6],
                                    z6[:, m * P:(m + 1) * P], ident6_s[:])
            nc.vector.tensor_copy(zT6[:, mr0 * 6:(mr0 + 16) * 6], pzt[:])

            zt_h = zT6[:].tensor
            def zsl(j0):        # [p, (m 16), (o 3)]
                return AP(zt_h, mr0 * 6 + j0, [[NT * 6, P], [6, 16], [1, 3]])
            iota_b = AP(iota[:].tensor, mr0, [[NT, P], [1, 16], [0, 3]])
            bo_b = AP(bo3_s[:].tensor, 0, [[3, P], [0, 16], [1, 3]])
            bm_b = AP(bm3_s[:].tensor, 0, [[3, P], [0, 16], [1, 3]])

            pos = pool.tile([P, 48], F32, tag=f"pos{h}", name=f"pos{h}")
            fl = pool.tile([P, 48], F32, tag=f"fl{h}", name=f"fl{h}")
            gt = pool.tile([P, 48], F32, tag=f"gt{h}", name=f"gt{h}")
            alp = pool.tile([P, 48], F32, tag=f"alp{h}", name=f"alp{h}")
            msk = pool.tile([P, 48], F32, tag=f"msk{h}", name=f"msk{h}")

            V = nc.vector
            V.tensor_tensor(pos[:], zsl(0), bo_b, A.add)
            V.tensor_tensor(pos[:], pos[:], iota_b, A.add)
            V.tensor_scalar(pos[:], pos[:], 0.0, float(L - 1), A.max, A.min)
            V.tensor_scalar(fl[:], pos[:], 8388608.0, 8388608.0, A.add, A.subtract)
            V.tensor_tensor(gt[:], fl[:], pos[:], A.is_gt)
            V.tensor_tensor(fl[:], fl[:], gt[:], A.subtract)
            V.tensor_tensor(alp[:], pos[:], fl[:], A.subtract)
            V.tensor_tensor(msk[:], zsl(3), bm_b, A.add)
            nc.scalar.activation(msk[:], msk[:],
                                 mybir.ActivationFunctionType.Sigmoid)
            # write dd/wc/wf straight into dwb as bf16, o-major [p, o*16+mc]
            db = dwb[h]
            def omaj(i):
                return AP(db[:].tensor, i * 48, [[144, P], [16, 3], [1, 16]])
            def iview(t):
                return AP(t[:].tensor, 0, [[48, P], [1, 3], [3, 16]])
            iota_o = AP(iota[:].tensor, mr0, [[NT, P], [0, 3], [1, 16]])
            V.tensor_tensor(omaj(0), iview(fl), iota_o, A.subtract)   # d
            V.tensor_tensor(omaj(2), iview(msk), iview(alp), A.mult)  # wc
            V.tensor_tensor(omaj(1), iview(msk), omaj(2), A.subtract) # wf

            # VV2: vv2[p, si*NT + (mr0+mc)] = sum of 6 contributions.
            E = nc.vector
            eqs = [pool.tile([P, ND * 16], BF16, tag=f"eq{h}_{i}",
                             name=f"eq{h}_{i}") for i in range(2)]
            db_h = db[:].tensor
            first = True
            for fc in range(2):          # 0: floor (wf), 1: ceil (wc)
                for o in range(3):
                    eq = eqs[(fc * 3 + o) % 2]
                    d_b = AP(db_h, o * 16, [[144, P], [0, ND], [1, 16]])
                    w_b = AP(db_h, (1 + fc) * 48 + o * 16,
                             [[144, P], [0, ND], [1, 16]])
                    sp_b = AP(spat_s[:].tensor, fc * 112,
                              [[2 * ND * 16, P], [16, ND], [1, 16]])
                    vv_v = AP(vv2[:].tensor, mr0, [[ND * NT, P], [NT, ND], [1, 16]])
                    eq_v = AP(eq[:].tensor, 0, [[ND * 16, P], [16, ND], [1, 16]])
                    E.tensor_tensor(eq_v, d_b, sp_b, A.is_equal)
                    if first:
                        E.tensor_tensor(vv_v, eq_v, w_b, A.mult)
                        first = False
                    else:
                        E.tensor_tensor(eq_v, eq_v, w_b, A.mult)
                        V.tensor_tensor(vv_v, vv_v, eq_v, A.add)

        # ---- shifts (2 chunks) then M2 build via local_scatter ----
        SUB = ((0, 8), (8, 15), (15, 24), (24, 32))
        pw = ps_sh.tile([P, ND * NT], F32, tag="pw")   # [p, u*NT + m]
        m2 = pool.tile([P, NT * F], BF16, tag="m2")    # m-major [p, m*134+f]

        def shifts_chunk(c):
            m0, m1 = (0, M0_END) if c == 0 else (M0_END, NT)
            for u in range(ND):
                si = 6 - u
                sh = u - 3
                dst = pw[:, u * NT + m0: u * NT + m1]
                main_rhs = vv2[:, si * NT + m0: si * NT + m1]
                if sh == 0:
                    nc.tensor.matmul(dst, sh_s[("m", u)][:], main_rhs,
                                     start=True, stop=True)
                elif sh > 0:
                    nc.tensor.matmul(dst, sh_s[("m", u)][:], main_rhs,
                                     start=True, stop=False)
                    ce = min(m1, NT - 1)   # carry: m gets vv2[m+1]
                    nc.tensor.matmul(pw[:, u * NT + m0: u * NT + ce],
                                     sh_s[("c", u)][:],
                                     vv2[:, si * NT + m0 + 1: si * NT + ce + 1],
                                     start=False, stop=True)
                else:
                    nc.tensor.matmul(dst, sh_s[("m", u)][:], main_rhs,
                                     start=True, stop=False)
                    cs = max(m0, 1)        # carry: m gets vv2[m-1]
                    nc.tensor.matmul(pw[:, u * NT + cs: u * NT + m1],
                                     sh_s[("c", u)][:],
                                     vv2[:, si * NT + cs - 1: si * NT + m1 - 1],
                                     start=False, stop=True)

        def m_subchunk(sc):
            m0, m1 = SUB[sc]
            nm = m1 - m0
            # w2m[p, (m-m0)*8 + u] = pw[p, u*NT + m]  (u=7 col is pad)
            wd = pool.tile([P, nm * 8], BF16, tag=f"w2m{sc}", name=f"w2m{sc}")
            src = AP(pw[:].tensor, m0, [[ND * NT, P], [1, nm], [NT, ND]])
            dst = AP(wd[:].tensor, 0, [[nm * 8, P], [8, nm], [1, ND]])
            nc.vector.tensor_copy(dst, src)
            nc.gpsimd.local_scatter(
                m2[:, m0 * F: m1 * F], wd[:], idxt_s[:, 0: nm * 8],
                channels=P, num_elems=nm * F, num_idxs=nm * 8)

        # ---- band: per (g, q) psum bank [128,512]; zero-init; spill + 4
        #      mains; copy into 1024-wide stage; DMA every 2 banks.
        #      bank q: l in [512q-3, 512q+509) ----
        bank_n = [0]
        store_n = [0]
        st_cur = [None, None]   # per g: current 1024-wide stage
        STORE_ENG = ("sync", "scalar", "sync", "scalar",
                     "gpsimd", "gpsimd", "sync", "scalar")

        def msl(m, f0, wid):    # m2 m-major view: tile m, cols [f0, f0+wid)
            return m2[:, m * F + f0: m * F + f0 + wid]

        def band_bank(g, q):
            bd = ps_bd.tile([P, 512], F32, tag="bd")
            nc.tensor.matmul(bd[:], xsl(4 * q, g), zeros_b[:],
                             start=True, stop=False, skip_group_check=True)
            if q > 0:
                m = 4 * q - 1
                nc.tensor.matmul(bd[:, 0:6], xsl(m, g), msl(m, 128, 6),
                                 start=False, stop=False, skip_group_check=True)
            for i in range(4):
                m = 4 * q + i
                wid = 134 if i < 3 else 128
                nc.tensor.matmul(bd[:, i * P: i * P + wid], xsl(m, g),
                                 msl(m, 0, wid),
                                 start=False, stop=(i == 3), skip_group_check=True)
            if q % 2 == 0:
                st = stg.tile([P, 1027], BF16, tag="st")
                st_cur[g] = st
            else:
                st = st_cur[g]
            half = (q % 2) * 512
            if bank_n[0] >= 14:
                nc.vector.tensor_copy(st[:, half:half + 512], bd[:])
            else:
                nc.scalar.copy(st[:, half:half + 512], bd[:])
            bank_n[0] += 1
            if q % 2 == 1 and q != 7:
                # pair store: out cols [1024q'-3, 1024q'+1021), q' = q//2
                qq = q // 2
                deng = getattr(nc, STORE_ENG[store_n[0]])
                store_n[0] += 1
                if qq == 0:
                    deng.dma_start(out_d[g * P:(g + 1) * P, 0:1021],
                                   st[:, 3:1024])
                else:
                    deng.dma_start(
                        out_d[g * P:(g + 1) * P, 1024 * qq - 3: 1024 * qq + 1021],
                        st[:, 0:1024])

        def band_tail(g):
            # out cols [4093, 4096) appended to the q'=3 pair stage; the
            # 1027-wide window [3069, 4096) stores as 2 halves on 2 queues
            bd = ps_bd.tile([P, 512], F32, tag="bd")
            m = NT - 1
            nc.tensor.matmul(bd[:, 0:6], xsl(m, g), msl(m, 128, 6),
                             start=True, stop=True, skip_group_check=True)
            st = st_cur[g]
            nc.vector.tensor_copy(st[:, 1024:1027], bd[:, 0:3])
            deng = getattr(nc, STORE_ENG[store_n[0]])
            store_n[0] += 1
            deng.dma_start(out_d[g * P:(g + 1) * P, 3069:4096], st[:])

        # ---- schedule ----
        tr_quad(0)
        for Q in range(1, 8):
            tr_quad(Q)
            conv_chunk(Q - 1)
            if Q == 4:
                half_front(0)   # zT-h0 enters PE stream after conv chunk 3
        conv_chunk(7)
        shifts_chunk(0)
        m_subchunk(0)           # m 0..7 (scatter on gpsimd, overlaps h1)
        m_subchunk(1)           # m 8..14 (shifts0 covers m 0..14)
        half_front(1)
        for g in range(2):      # banks 0..2: m 0..11 (+spills)
            for q in range(3):
                band_bank(g, q)
        shifts_chunk(1)         # needs vv2 h1 (DVE done by now)
        m_subchunk(2)           # m 15..23
        for g in range(2):
            for q in range(3, 6):   # m 12..23 + spills
                band_bank(g, q)
        m_subchunk(3)           # m 24..31
        for g in range(2):
            for q in range(6, 8):
                band_bank(g, q)
            band_tail(g)

    nc.compile()
    return nc


def _get_nc(w_off, b_off, w_mask, b_mask):
    key = (w_off.tobytes(), b_off.tobytes(), w_mask.tobytes(), b_mask.tobytes())
    if key not in _cache:
        _cache[key] = _build(w_off, b_off, w_mask, b_mask)
    return _cache[key]


def kernel(x, w_off, b_off, w_mask, b_mask):
    x = np.ascontiguousarray(np.asarray(x, dtype=np.float32))
    nc = _get_nc(np.asarray(w_off, np.float32), np.asarray(b_off, np.float32),
                 np.asarray(w_mask, np.float32), np.asarray(b_mask, np.float32))
    in_maps = [{"x": x[b]} for b in range(B)]
    res = run_bass_kernel_spmd(nc, in_maps, list(range(B)))
    # out_d is the (C, L) bf16 buffer; reference returns its raw (L, C) reshape
    return np.stack([np.asarray(res.results[b]["out"]).reshape(L, C)
                     .astype(np.float32) for b in range(B)])
